# revision 59
# baseline (speedup 1.0000x reference)
"""Multi-head self-attention (BERT-style) Trainium2 kernel.

Sharding: 8 cores = 2 batches x 4 head-groups (3 heads each).
Per core (batch b, heads h0..h2):
  Q^T/K^T = W^T X^T per head, duplicated across both partition halves
            (score matmuls contract K=128; the 2x is folded into scale —
            full-tile matmuls stream ~2.4x faster per instruction than
            tile_position K=64 matmuls on this hardware)
  V       = X Wv (+bias) with ones-column per head (denominator trick)
  scores  = K-tile x Q^T -> [128 keys, 1024 q] PSUM tiles (2 banks),
            exp on ScalarE with attention-mask as per-partition bias,
            ONE activation instruction per 1024 q-columns (amortizes the
            ~222-cycle ACT fixed cost; ACT is the throughput bottleneck)
  ctx/denom via PV matmuls accumulating [65, 1024] PSUM ctx halves
  normalize: denom copy -> reciprocal -> partition_broadcast -> mult,
             chunked per 512 cols for pipelined latency
  out partial = ctx01^T Wo[0:128] + ctx2d^T Wo2dup (rows pre-halved)
Host sums the 4 partials per batch and adds bo.

Schedule: head 0 runs with its q-half-1 score tiles trailing half-0 by 3
key-chunks (so Q-tiles 2,3 can be emitted as filler), V/K/Q projections
interleave into the stream to keep ScalarE fed; head 1 is c-outer with
head-2's Q/K projections spread through it; head 2 is q-half-major so
its first-half normalize lands early and the output projection for
q-tiles 0-7 overlaps head 2's second half (reusing the freed ctx PSUM
slot). PSUM: 2x[128,1024] score slots + 2x[65,1024] ctx slots = 8 banks.
"""

import sys

sys.path.insert(0, "/opt/trn_rl_repo")

from contextlib import ExitStack

import numpy as np

import concourse.bass as bass
import concourse.mybir as mybir
import concourse.tile as tile
from concourse import bacc
from concourse.bass_utils import run_bass_kernel_spmd

F16 = mybir.dt.float16
F32 = mybir.dt.float32

H = 768
NH = 12
HD = 64
B = 2
S = 2048
HC = H // 128  # 6 h-chunks of 128
KT = S // 128  # 16 k-tiles of 128
D3 = 3 * HD  # 192 cols per core
N_CORES = 8


def build_kernel():
    nc = bacc.Bacc(
        "TRN2",
        target_bir_lowering=False,
        debug=False,
        enable_asserts=False,
        num_devices=N_CORES,
    )

    # All inputs partition-major so every DMA descriptor carries a full
    # (multi-)KB partition line: DMA engines pay ~220ns fixed per
    # descriptor, so 4KB-row transfers run at half the bus rate.
    xt = nc.dram_tensor("xt", [128, HC * S], F16, kind="ExternalInput")
    wqk = nc.dram_tensor("wqk", [128, 2 * HC * D3], F16, kind="ExternalInput")
    wvb = nc.dram_tensor("wvb", [128, HC * (D3 + 128)], F16, kind="ExternalInput")
    wo = nc.dram_tensor("wo", [128, 2 * H], F16, kind="ExternalInput")
    bqkm = nc.dram_tensor("bqkm", [128, 20], F32, kind="ExternalInput")
    bv = nc.dram_tensor("bv", [1, D3], F16, kind="ExternalInput")
    out = nc.dram_tensor("out", [S, H], F16, kind="ExternalOutput")

    with tile.TileContext(nc) as tc:
        _emit(tc, xt, wqk, wvb, wo, bqkm, bv, out)

    nc.compile()
    return nc


def _emit(tc, xt, wqk, wvb, wo, bqkm, bv, out):
    nc = tc.nc
    ADD = mybir.AluOpType.add
    MULT = mybir.AluOpType.mult
    EXP = mybir.ActivationFunctionType.Exp

    with ExitStack() as stack:
        persist = stack.enter_context(tc.tile_pool(name="persist", bufs=1))

        # ---- constant / persistent SBUF tiles (blob DMAs + views) ----
        xt_sb = persist.tile([128, HC, S], F16)
        wqk_sb = persist.tile([128, 2, HC, D3], F16)
        wq_sb, wk_sb = wqk_sb[:, 0], wqk_sb[:, 1]
        wvb_sb = persist.tile([128, HC, D3 + 128], F16)
        wv_sb, wb2_sb = wvb_sb[:, :, 0:D3], wvb_sb[:, :, D3:]
        woo_sb = persist.tile([128, 2 * H], F16)
        wo_sb, wo2d = woo_sb[:, 0:H], woo_sb[:, H:]
        bqkm_sb = persist.tile([128, 20], F32)
        bq_sb, bk_sb = bqkm_sb[:, 0:2], bqkm_sb[:, 2:4]
        mask_sb = bqkm_sb[:, 4:20]
        bv_sb = persist.tile([1, D3], F16)

        # xt in three serial 2-chunk DMAs (8KB/partition descriptors) on
        # ONE queue: concurrent queues split the 16 DMA engines, so
        # serializing gets the first pair (and with it the projection
        # pipeline) started earliest. Weights stream on the scalar queue.
        for i in range(3):
            nc.sync.dma_start(
                xt_sb[:, 2 * i : 2 * i + 2, :].rearrange("p c s -> p (c s)"),
                xt.ap()[:, 2 * i * S : (2 * i + 2) * S],
            )
        nc.scalar.dma_start(
            wqk_sb[:, 0].rearrange("p c d -> p (c d)"), wqk.ap()[:, 0 : HC * D3]
        )
        nc.scalar.dma_start(
            wqk_sb[:, 1].rearrange("p c d -> p (c d)"), wqk.ap()[:, HC * D3 :]
        )
        nc.scalar.dma_start(bqkm_sb[:], bqkm.ap())
        nc.scalar.dma_start(bv_sb[:], bv.ap())
        # wv/wb2/wo are needed only after the first scores: keep them
        # behind the critical xt/wqk traffic.
        nc.scalar.dma_start(wvb_sb[:].rearrange("p c d -> p (c d)"), wvb.ap())
        nc.scalar.dma_start(woo_sb[:], wo.ap())
        bv_bc = persist.tile([128, D3], F16)
        nc.gpsimd.partition_broadcast(bv_bc[:], bv_sb[:])
        # warm the ACT exp table during the DMA lead-in
        warm = persist.tile([1, 8], F32)
        nc.vector.memset(warm[:], 0.0)
        nc.scalar.activation(warm[:], warm[:], EXP)

        # Q^T/K^T per head, duplicated across both partition halves
        qd = [persist.tile([128, S], F16, name=f"qd{h}") for h in range(3)]
        kd = [persist.tile([128, S], F16, name=f"kd{h}") for h in range(3)]
        # V: [k, 3*(64+1)] with a ones column per head (col 64 of each 65)
        v_sb = persist.tile([128, KT, 3 * 65], F16)
        for h in range(3):
            nc.vector.memset(
                v_sb[:].rearrange("p k (h x) -> p k h x", x=65)[:, :, h, 64:65], 1.0
            )
        # normalized context: heads 0,1 stacked; head 2 duplicated
        ctx01 = persist.tile([128, S], F16)
        ctx2d = persist.tile([128, S], F16)
        ctx_tmp = persist.tile([64, S], F16)

        # ---- PSUM: 3x [128,1024] score/work slots (12KB) + 2x [65,512]
        # ctx quarter slots (4KB) = exactly 8 banks. The 3-deep score ring
        # lets the PE run two exp tiles ahead of ScalarE.
        ctx_pool = tc.alloc_tile_pool(name="ctx_ps", bufs=2, space="PSUM")
        work = tc.alloc_tile_pool(name="work", bufs=3, space="PSUM")
        p_pool = stack.enter_context(tc.tile_pool(name="p_sb", bufs=30))
        norm_pool = stack.enter_context(tc.tile_pool(name="norm", bufs=4))
        ob_pool = stack.enter_context(tc.tile_pool(name="ob", bufs=6))

        def emit_qk(w_sb, dst, b_sb, qt, bgroup):
            """One [128, 512] projection tile + bias + partition-dup DMAs."""
            qs = slice(qt * 512, (qt + 1) * 512)
            pq = work.tile([128, 512], F32, tag="wk", name="pq")
            for hc in range(HC):
                nc.tensor.matmul(
                    pq[:],
                    lhsT=w_sb[:, hc, 0:128],
                    rhs=xt_sb[:, hc, qs],
                    start=(hc == 0),
                    stop=(hc == HC - 1),
                )
            if bgroup:
                # rows 0:64 = Q2, rows 64:128 = K2 (w_sb is [Wq2 | Wk2])
                nc.vector.tensor_scalar(
                    qd[2][0:64, qs], pq[0:64, :], b_sb[0:64, 1:2], None, ADD
                )
                nc.vector.tensor_scalar(
                    kd[2][64:128, qs], pq[64:128, :], b_sb[64:128, 1:2], None, ADD
                )
                nc.sync.dma_start(qd[2][64:128, qs], qd[2][0:64, qs])
                nc.sync.dma_start(kd[2][0:64, qs], kd[2][64:128, qs])
            else:
                nc.vector.tensor_scalar(
                    dst[0][0:64, qs], pq[0:64, :], b_sb[0:64, 0:1], None, ADD
                )
                nc.vector.tensor_scalar(
                    dst[1][64:128, qs], pq[64:128, :], b_sb[64:128, 0:1], None, ADD
                )
                nc.sync.dma_start(dst[0][64:128, qs], dst[0][0:64, qs])
                nc.sync.dma_start(dst[1][0:64, qs], dst[1][64:128, qs])

        def emit_v(kt):
            ks = slice(kt * 128, (kt + 1) * 128)
            pv = work.tile([128, D3], F32, tag="wk", name="pv")
            for hc in range(HC):
                nc.tensor.matmul(
                    pv[:],
                    lhsT=xt_sb[:, hc, ks],
                    rhs=wv_sb[:, hc, :],
                    start=(hc == 0),
                    stop=(hc == HC - 1),
                )
            nc.vector.tensor_tensor(
                v_sb[:].rearrange("p k (h x) -> p k h x", x=65)[:, kt, :, 0:64],
                pv[:].rearrange("p (h x) -> p h x", x=64),
                bv_bc[:].rearrange("p (h x) -> p h x", x=64),
                ADD,
            )
            v_done.add(kt)

        # ---- score/exp/PV pipeline ----
        # pv_q holds exp'd tiles; each pop issues the 2 PV matmuls for one
        # [128,1024] prob tile. Pops are RATE-scheduled per phase: PE-bound
        # phases (head-0 with its V/proj fillers) defer PV work into the
        # ACT-bound phases' slack, balancing engine load across the run.
        pv_q = []
        pop_state = {"budget": 0.0, "rate": 0.0, "suppress": 0}
        v_done = set()

        def pop_pv():
            h, half, c, cq, pt = pv_q.pop(0)
            for j in range(2):
                nc.tensor.matmul(
                    cq[j][:],
                    lhsT=v_sb[:, c, h * 65 : (h + 1) * 65],
                    rhs=pt[:, j * 512 : (j + 1) * 512],
                    start=(c == 0),
                    stop=(c == KT - 1),
                )
            if c == KT - 1:
                emit_normalize(h, half, cq)
                # a few pop-free pushes so the next quarter's PVs don't
                # reach the PE before the normalize chain releases the
                # ctx slot (WAR on the in-order queue)
                pop_state["suppress"] = 3

        def emit_score_tile(h, half, c, cq):
            """[128 keys, 1024 q] scores -> exp -> queue PV."""
            ks = slice(c * 128, (c + 1) * 128)
            sc = work.tile([128, 1024], F32, tag="wk", name="sc")
            for j in range(2):
                qs = slice(half * 1024 + j * 512, half * 1024 + (j + 1) * 512)
                nc.tensor.matmul(
                    sc[:, j * 512 : (j + 1) * 512],
                    lhsT=kd[h][:, ks],
                    rhs=qd[h][:, qs],
                    start=True,
                    stop=True,
                )
            pt = p_pool.tile([128, 1024], F16, tag="pt")
            nc.scalar.activation(
                pt[:], sc[:], EXP, bias=mask_sb[:, c : c + 1], scale=1.0
            )
            pv_q.append((h, half, c, cq, pt))
            st = pop_state
            st["budget"] += st["rate"]
            if st["suppress"] > 0:
                st["suppress"] -= 1
                return
            while st["budget"] >= 1.0 and pv_q and st["suppress"] == 0:
                # never emit a PV before its V tile exists
                if pv_q[0][2] not in v_done:
                    break
                pop_pv()
                st["budget"] -= 1.0
            # pt-ring safety: force-drain if the backlog nears the ring size
            while len(pv_q) > 26 and pv_q[0][2] in v_done:
                pop_pv()

        def emit_normalize(h, half, cq):
            """denom -> recip -> broadcast -> mult, stage-major over the
            half's two 512-col ctx quarters."""
            base = half * 1024
            if h == 0:
                dst = ctx01[0:64, :]
            elif h == 1:
                dst = ctx_tmp[:]
            else:
                dst = ctx2d[0:64, :]
            denoms, recips, rbcs = [], [], []
            for j in range(2):
                denom = norm_pool.tile([1, 512], F32, tag="denom")
                if h == 2 and half == 1:
                    # ACT is idle after the final exp; keep DVE free for
                    # the tail's output casts
                    nc.scalar.copy(denom[:], cq[j][64:65, :])
                else:
                    nc.vector.tensor_copy(denom[:], cq[j][64:65, :])
                denoms.append(denom)
            for j in range(2):
                recip = norm_pool.tile([1, 512], F32, tag="recip")
                nc.vector.reciprocal_approx_fast(recip[:], denoms[j][:])
                recips.append(recip)
            for j in range(2):
                rbc = norm_pool.tile([64, 512], F32, tag="rbc")
                nc.gpsimd.partition_broadcast(rbc[:], recips[j][:])
                rbcs.append(rbc)
            for j in range(2):
                dcols = slice(base + j * 512, base + (j + 1) * 512)
                nc.vector.tensor_tensor(
                    dst[:, dcols], cq[j][0:64, :], rbcs[j][:], MULT
                )
            if h == 1:
                nc.gpsimd.dma_start(
                    ctx01[64:128, base : base + 1024], ctx_tmp[:, base : base + 1024]
                )
            elif h == 2:
                nc.gpsimd.dma_start(
                    ctx2d[64:128, base : base + 1024], ctx2d[0:64, base : base + 1024]
                )

        def emit_outproj(qt, po, cast_eng, dma_eng):
            """po = ctx01^T Wo01 + ctx2d^T Wo2dup for one 128-row q-tile."""
            qs = slice(qt * 128, (qt + 1) * 128)
            # matmul dest is capped at 512 fp32 elements (one PSUM bank)
            for ns, ne in ((0, 512), (512, 768)):
                nc.tensor.matmul(
                    po[:, ns:ne],
                    lhsT=ctx01[:, qs],
                    rhs=wo_sb[:, ns:ne],
                    start=True,
                    stop=False,
                )
                nc.tensor.matmul(
                    po[:, ns:ne],
                    lhsT=ctx2d[:, qs],
                    rhs=wo2d[:, ns:ne],
                    start=False,
                    stop=True,
                )
            ob = ob_pool.tile([128, H], F16, tag="ob")
            if cast_eng == "v":
                nc.vector.tensor_copy(ob[:], po[:])
            elif cast_eng == "a":
                nc.scalar.copy(ob[:], po[:])
            else:
                nc.vector.tensor_copy(ob[:, 0:384], po[:, 0:384])
                nc.scalar.copy(ob[:, 384:768], po[:, 384:768])
            dma_eng.dma_start(out.ap()[qs, :], ob[:])

        # ---------------- emission schedule ----------------
        # Lead-in: Q0/K0/Q1 projections interleaved per h-chunk (each mm is
        # gated by its xt-chunk DMA, so all three tiles finish ~together at
        # xt-complete). Q1 borrows a then-idle ctx-pool PSUM slot.
        pq1 = ctx_pool.tile([128, 512], F32, tag="ctx", name="pq1")
        pq0 = work.tile([128, 512], F32, tag="wk", name="pq0")
        pk0 = work.tile([128, 512], F32, tag="wk", name="pk0")
        lead = ((pq0, wq_sb, 0), (pk0, wk_sb, 0), (pq1, wq_sb, 1))
        for hc in range(HC):
            for pt_, w_sb, qt in lead:
                nc.tensor.matmul(
                    pt_[:],
                    lhsT=w_sb[:, hc, 0:128],
                    rhs=xt_sb[:, hc, qt * 512 : (qt + 1) * 512],
                    start=(hc == 0),
                    stop=(hc == HC - 1),
                )

        for pt_, dst, b_sb, qt in (
            (pq0, qd, bq_sb, 0),
            (pk0, kd, bk_sb, 0),
            (pq1, qd, bq_sb, 1),
        ):
            qs = slice(qt * 512, (qt + 1) * 512)
            nc.vector.tensor_scalar(
                dst[0][0:64, qs], pt_[0:64, :], b_sb[0:64, 0:1], None, ADD
            )
            nc.vector.tensor_scalar(
                dst[1][64:128, qs], pt_[64:128, :], b_sb[64:128, 0:1], None, ADD
            )
            # sync queue: gpsimd is still streaming xt chunks 2-3 here
            nc.sync.dma_start(dst[0][64:128, qs], dst[0][0:64, qs])
            nc.sync.dma_start(dst[1][0:64, qs], dst[1][64:128, qs])

        def ctx_quarters(name):
            return (
                ctx_pool.tile([65, 512], F32, tag="ctx", name=f"{name}q0"),
                ctx_pool.tile([65, 512], F32, tag="ctx", name=f"{name}q1"),
            )
        # Strictly half-major phases (h, half); each holds its two ctx
        # quarter tiles. Per-phase PV pop rates spread the PV backlog into
        # the ACT-bound phases' PE slack: head-0 half-0 (scores + Q/K proj)
        # runs ACT-bound with no PV/V work; V moves to half 1; the backlog
        # drains hardest in the pure phases (1,0)/(2,0).
        h0_pre = {3: 1, 6: 2, 10: 3}  # K-tile qt before its first use
        h0_post = {0: 2, 1: 3}  # Q-tile qt right after c0/c1
        pop_state["rate"] = 0.0
        cq = ctx_quarters("c00")
        for c in range(KT):
            if c in h0_pre:
                emit_qk(wk_sb, kd, bk_sb, h0_pre[c], False)
            emit_score_tile(0, 0, c, cq)
            if c in h0_post:
                emit_qk(wq_sb, qd, bq_sb, h0_post[c], False)

        pop_state["rate"] = 0.4
        cq = ctx_quarters("c01")
        for c in range(KT):
            emit_v(c)
            emit_score_tile(0, 1, c, cq)

        pop_state["rate"] = 1.8
        cq = ctx_quarters("c10")
        for c in range(KT):
            emit_score_tile(1, 0, c, cq)

        pop_state["rate"] = 1.0
        cq = ctx_quarters("c11")
        for c in range(KT):
            if c in (1, 5, 9, 13):
                emit_qk(wb2_sb, None, bq_sb, (c - 1) // 4, True)
            emit_score_tile(1, 1, c, cq)

        pop_state["rate"] = 1.8
        cq = ctx_quarters("c20")
        for c in range(KT):
            emit_score_tile(2, 0, c, cq)

        pop_state["rate"] = 1.0
        cq = ctx_quarters("c21")
        po_a = None
        for c in range(KT):
            emit_score_tile(2, 1, c, cq)
            if c == 7:
                po_a = work.tile([128, H], F32, tag="wk", name="po_a")
            if c >= 8:
                emit_outproj(
                    c - 8,
                    po_a,
                    "v" if c < 14 else "s",
                    nc.sync if c % 2 == 0 else nc.gpsimd,
                )
        while pv_q:
            pop_pv()

        # tail: q-tiles 8-15, rotating po through freed work-ring slots
        po_b = work.tile([128, H], F32, tag="wk", name="po_b")
        po_c = work.tile([128, H], F32, tag="wk", name="po_c")
        for i, qt in enumerate(range(8, KT)):
            po = (po_b, po_c, po_a)[i % 3]
            emit_outproj(qt, po, "s", nc.sync if i % 2 else nc.scalar)
        work.release()
        ctx_pool.release()


_NC_CACHE = None


def _get_nc():
    global _NC_CACHE
    if _NC_CACHE is None:
        _NC_CACHE = build_kernel()
    return _NC_CACHE


def _pack_w(w):
    """[768, 192] -> [128, 6*192] with row p = concat_c w[c*128+p, :]."""
    return np.ascontiguousarray(
        w.reshape(HC, 128, D3).transpose(1, 0, 2).reshape(128, HC * D3)
    )


def make_in_maps(hidden_states, attention_mask, Wq, bq, Wk, bk, Wv, bv, Wo, bo):
    hidden_states = np.asarray(hidden_states, np.float32)
    attention_mask = np.asarray(attention_mask, np.float32)
    Wq = np.asarray(Wq, np.float32)
    Wk = np.asarray(Wk, np.float32)
    Wv = np.asarray(Wv, np.float32)
    Wo = np.asarray(Wo, np.float32)
    bq = np.asarray(bq, np.float32)
    bk = np.asarray(bk, np.float32)
    bv = np.asarray(bv, np.float32)

    scale = 0.5 / np.sqrt(np.float32(HD))  # extra 1/2: scores use dup-row K=128
    in_maps = []
    for core in range(N_CORES):
        b, g = divmod(core, 4)
        cols = slice(D3 * g, D3 * (g + 1))
        bq_s = (bq[cols] * scale).astype(np.float32)
        bk_s = bk[cols].astype(np.float32)
        bq_pack = np.zeros((2, 128), np.float32)
        bq_pack[0] = bq_s[0:128]
        bq_pack[1, 0:64] = bq_s[128:192]
        bq_pack[1, 64:128] = bk_s[128:192]
        bk_pack = np.zeros((2, 128), np.float32)
        bk_pack[0] = bk_s[0:128]

        # xt partition-major: row p = concat_c X^T[c*128+p, :]
        xtp = (
            np.ascontiguousarray(hidden_states[b].T)
            .astype(np.float16)
            .reshape(HC, 128, S)
            .transpose(1, 0, 2)
            .reshape(128, HC * S)
        )
        wq_p = _pack_w((Wq[:, cols] * scale).astype(np.float16))
        wk_p = _pack_w(Wk[:, cols].astype(np.float16))
        wv_p = _pack_w(Wv[:, cols].astype(np.float16))
        wb2_p = (
            np.concatenate(
                [Wq[:, cols][:, 128:192] * scale, Wk[:, cols][:, 128:192]], axis=1
            )
            .astype(np.float16)
            .reshape(HC, 128, 128)
            .transpose(1, 0, 2)
            .reshape(128, HC * 128)
        )
        # wv|wb2 interleaved per h-chunk: [128, HC, 192+128]
        wvb_p = np.concatenate(
            [wv_p.reshape(128, HC, D3), wb2_p.reshape(128, HC, 128)], axis=2
        ).reshape(128, HC * (D3 + 128))
        wo2h = (Wo[cols, :][128:192] * 0.5).astype(np.float16)
        woo = np.concatenate(
            [
                Wo[cols, :][0:128].astype(np.float16),
                np.concatenate([wo2h, wo2h], axis=0),
            ],
            axis=1,
        )
        bqkm_p = np.concatenate(
            [
                bq_pack.T,
                bk_pack.T,
                attention_mask[b, 0, 0, :].reshape(KT, 128).T,
            ],
            axis=1,
        ).astype(np.float32)
        in_maps.append(
            {
                "xt": np.ascontiguousarray(xtp),
                "wqk": np.ascontiguousarray(np.concatenate([wq_p, wk_p], axis=1)),
                "wvb": np.ascontiguousarray(wvb_p),
                "wo": np.ascontiguousarray(woo),
                "bqkm": np.ascontiguousarray(bqkm_p),
                "bv": bv[cols].reshape(1, D3).astype(np.float16),
            }
        )
    return in_maps


def assemble_out(results, bo):
    out = np.zeros((B, S, H), np.float32)
    for core in range(N_CORES):
        b = core // 4
        out[b] += results[core]["out"].astype(np.float32)
    out += np.asarray(bo, np.float32)
    return out


def kernel(hidden_states, attention_mask, Wq, bq, Wk, bk, Wv, bv, Wo, bo):
    in_maps = make_in_maps(
        hidden_states, attention_mask, Wq, bq, Wk, bk, Wv, bv, Wo, bo
    )
    res = run_bass_kernel_spmd(_get_nc(), in_maps, list(range(N_CORES)))
    return assemble_out(res.results, bo)


# revision 65
# speedup vs baseline: 1.0660x; 1.0660x over previous
"""Multi-head self-attention (BERT-style) Trainium2 kernel.

Sharding: 8 cores = 2 batches x 4 head-groups (3 heads each).
Per core (batch b, heads h0..h2):
  Q^T/K^T = W^T X^T per head, duplicated across both partition halves
            (score matmuls contract K=128; the 2x is folded into scale —
            full-tile matmuls stream ~2.4x faster per instruction than
            tile_position K=64 matmuls on this hardware)
  V       = X Wv (+bias) with ones-column per head (denominator trick)
  scores  = K-tile x Q^T -> [128 keys, 1024 q] PSUM tiles (2 banks),
            exp on ScalarE with attention-mask as per-partition bias,
            ONE activation instruction per 1024 q-columns (amortizes the
            ~222-cycle ACT fixed cost; ACT is the throughput bottleneck)
  ctx/denom via PV matmuls accumulating [65, 1024] PSUM ctx halves
  normalize: denom copy -> reciprocal -> partition_broadcast -> mult,
             chunked per 512 cols for pipelined latency
  out partial = ctx01^T Wo[0:128] + ctx2d^T Wo2dup (rows pre-halved)
Host sums the 4 partials per batch and adds bo.

Schedule: head 0 runs with its q-half-1 score tiles trailing half-0 by 3
key-chunks (so Q-tiles 2,3 can be emitted as filler), V/K/Q projections
interleave into the stream to keep ScalarE fed; head 1 is c-outer with
head-2's Q/K projections spread through it; head 2 is q-half-major so
its first-half normalize lands early and the output projection for
q-tiles 0-7 overlaps head 2's second half (reusing the freed ctx PSUM
slot). PSUM: 2x[128,1024] score slots + 2x[65,1024] ctx slots = 8 banks.
"""

import sys

sys.path.insert(0, "/opt/trn_rl_repo")

from contextlib import ExitStack

import numpy as np

import concourse.bass as bass
import concourse.mybir as mybir
import concourse.tile as tile
from concourse import bacc
from concourse.bass_utils import run_bass_kernel_spmd

F16 = mybir.dt.float16
F32 = mybir.dt.float32

H = 768
NH = 12
HD = 64
B = 2
S = 2048
HC = H // 128  # 6 h-chunks of 128
KT = S // 128  # 16 k-tiles of 128
D3 = 3 * HD  # 192 cols per core
N_CORES = 8


def build_kernel():
    nc = bacc.Bacc(
        "TRN2",
        target_bir_lowering=False,
        debug=False,
        enable_asserts=False,
        num_devices=N_CORES,
    )

    # All inputs partition-major so every DMA descriptor carries a full
    # (multi-)KB partition line: DMA engines pay ~220ns fixed per
    # descriptor, so 4KB-row transfers run at half the bus rate.
    xt = nc.dram_tensor("xt", [128, HC * S], F16, kind="ExternalInput")
    wqk = nc.dram_tensor("wqk", [128, 2 * HC * D3], F16, kind="ExternalInput")
    wvb = nc.dram_tensor("wvb", [128, HC * (D3 + 128)], F16, kind="ExternalInput")
    wo = nc.dram_tensor("wo", [128, 2 * H], F16, kind="ExternalInput")
    bqkm = nc.dram_tensor("bqkm", [128, 20], F32, kind="ExternalInput")
    bv = nc.dram_tensor("bv", [1, D3], F16, kind="ExternalInput")
    out = nc.dram_tensor("out", [S, H], F16, kind="ExternalOutput")

    with tile.TileContext(nc) as tc:
        _emit(tc, xt, wqk, wvb, wo, bqkm, bv, out)

    nc.compile()
    return nc


def _emit(tc, xt, wqk, wvb, wo, bqkm, bv, out):
    nc = tc.nc
    ADD = mybir.AluOpType.add
    MULT = mybir.AluOpType.mult
    EXP = mybir.ActivationFunctionType.Exp

    with ExitStack() as stack:
        persist = stack.enter_context(tc.tile_pool(name="persist", bufs=1))

        # ---- constant / persistent SBUF tiles (blob DMAs + views) ----
        xt_sb = persist.tile([128, HC, S], F16)
        wqk_sb = persist.tile([128, 2, HC, D3], F16)
        wq_sb, wk_sb = wqk_sb[:, 0], wqk_sb[:, 1]
        wvb_sb = persist.tile([128, HC, D3 + 128], F16)
        wv_sb, wb2_sb = wvb_sb[:, :, 0:D3], wvb_sb[:, :, D3:]
        woo_sb = persist.tile([128, 2 * H], F16)
        wo_sb, wo2d = woo_sb[:, 0:H], woo_sb[:, H:]
        bqkm_sb = persist.tile([128, 20], F32)
        bq_sb, bk_sb = bqkm_sb[:, 0:2], bqkm_sb[:, 2:4]
        mask_sb = bqkm_sb[:, 4:20]
        bv_sb = persist.tile([1, D3], F16)

        # xt in three serial 2-chunk DMAs (8KB/partition descriptors) on
        # ONE queue: concurrent queues split the 16 DMA engines, so
        # serializing gets the first pair (and with it the projection
        # pipeline) started earliest. Weights stream on the scalar queue.
        for i in range(3):
            nc.sync.dma_start(
                xt_sb[:, 2 * i : 2 * i + 2, :].rearrange("p c s -> p (c s)"),
                xt.ap()[:, 2 * i * S : (2 * i + 2) * S],
            )
        nc.scalar.dma_start(
            wqk_sb[:, 0].rearrange("p c d -> p (c d)"), wqk.ap()[:, 0 : HC * D3]
        )
        nc.scalar.dma_start(
            wqk_sb[:, 1].rearrange("p c d -> p (c d)"), wqk.ap()[:, HC * D3 :]
        )
        nc.scalar.dma_start(bqkm_sb[:], bqkm.ap())
        nc.scalar.dma_start(bv_sb[:], bv.ap())
        # wv/wb2/wo are needed only after the first scores: keep them
        # behind the critical xt/wqk traffic.
        nc.scalar.dma_start(wvb_sb[:].rearrange("p c d -> p (c d)"), wvb.ap())
        nc.scalar.dma_start(woo_sb[:], wo.ap())
        bv_bc = persist.tile([128, D3], F16)
        nc.gpsimd.partition_broadcast(bv_bc[:], bv_sb[:])
        # warm the ACT exp table during the DMA lead-in
        warm = persist.tile([1, 8], F32)
        nc.vector.memset(warm[:], 0.0)
        nc.scalar.activation(warm[:], warm[:], EXP)

        # Q^T/K^T per head, duplicated across both partition halves
        qd = [persist.tile([128, S], F16, name=f"qd{h}") for h in range(3)]
        kd = [persist.tile([128, S], F16, name=f"kd{h}") for h in range(3)]
        # V: [k, 3*(64+1)] with a ones column per head (col 64 of each 65)
        v_sb = persist.tile([128, KT, 3 * 65], F16)
        for h in range(3):
            nc.vector.memset(
                v_sb[:].rearrange("p k (h x) -> p k h x", x=65)[:, :, h, 64:65], 1.0
            )
        # normalized context: heads 0,1 stacked; head 2 duplicated
        ctx01 = persist.tile([128, S], F16)
        ctx2d = persist.tile([128, S], F16)
        ctx_tmp = persist.tile([64, S], F16)

        # ---- PSUM: 3x [128,1024] score/work slots (12KB) + 2x [65,512]
        # ctx quarter slots (4KB) = exactly 8 banks. The 3-deep score ring
        # lets the PE run two exp tiles ahead of ScalarE.
        ctx_pool = tc.alloc_tile_pool(name="ctx_ps", bufs=2, space="PSUM")
        work = tc.alloc_tile_pool(name="work", bufs=3, space="PSUM")
        p_pool = stack.enter_context(tc.tile_pool(name="p_sb", bufs=8))
        norm_pool = stack.enter_context(tc.tile_pool(name="norm", bufs=4))
        ob_pool = stack.enter_context(tc.tile_pool(name="ob", bufs=6))

        def emit_qk(w_sb, dst, b_sb, qt, bgroup):
            """One [128, 512] projection tile + bias + partition-dup DMAs."""
            qs = slice(qt * 512, (qt + 1) * 512)
            pq = work.tile([128, 512], F32, tag="wk", name="pq")
            for hc in range(HC):
                nc.tensor.matmul(
                    pq[:],
                    lhsT=w_sb[:, hc, 0:128],
                    rhs=xt_sb[:, hc, qs],
                    start=(hc == 0),
                    stop=(hc == HC - 1),
                )
            if bgroup:
                # rows 0:64 = Q2, rows 64:128 = K2 (w_sb is [Wq2 | Wk2])
                nc.vector.tensor_scalar(
                    qd[2][0:64, qs], pq[0:64, :], b_sb[0:64, 1:2], None, ADD
                )
                nc.vector.tensor_scalar(
                    kd[2][64:128, qs], pq[64:128, :], b_sb[64:128, 1:2], None, ADD
                )
                nc.sync.dma_start(qd[2][64:128, qs], qd[2][0:64, qs])
                nc.sync.dma_start(kd[2][0:64, qs], kd[2][64:128, qs])
            else:
                nc.vector.tensor_scalar(
                    dst[0][0:64, qs], pq[0:64, :], b_sb[0:64, 0:1], None, ADD
                )
                nc.vector.tensor_scalar(
                    dst[1][64:128, qs], pq[64:128, :], b_sb[64:128, 0:1], None, ADD
                )
                nc.sync.dma_start(dst[0][64:128, qs], dst[0][0:64, qs])
                nc.sync.dma_start(dst[1][0:64, qs], dst[1][64:128, qs])

        def emit_v(kt):
            ks = slice(kt * 128, (kt + 1) * 128)
            pv = work.tile([128, D3], F32, tag="wk", name="pv")
            for hc in range(HC):
                nc.tensor.matmul(
                    pv[:],
                    lhsT=xt_sb[:, hc, ks],
                    rhs=wv_sb[:, hc, :],
                    start=(hc == 0),
                    stop=(hc == HC - 1),
                )
            nc.vector.tensor_tensor(
                v_sb[:].rearrange("p k (h x) -> p k h x", x=65)[:, kt, :, 0:64],
                pv[:].rearrange("p (h x) -> p h x", x=64),
                bv_bc[:].rearrange("p (h x) -> p h x", x=64),
                ADD,
            )

        # ---- score/exp/PV pipeline ----
        # pv_q holds exp'd tiles; each pop issues the 2 PV matmuls for one
        # [128,1024] prob tile. The lag keeps the in-order PE from stalling
        # on a not-yet-finished exp, and carries ctx WAR slack across
        # head/half boundaries (ctx ring bufs=2).
        pv_q = []
        PV_LAG = 4

        def pop_pv():
            h, half, c, cq, pt = pv_q.pop(0)
            for j in range(2):
                nc.tensor.matmul(
                    cq[j][:],
                    lhsT=v_sb[:, c, h * 65 : (h + 1) * 65],
                    rhs=pt[:, j * 512 : (j + 1) * 512],
                    start=(c == 0),
                    stop=(c == KT - 1),
                )
            if c == KT - 1:
                emit_normalize(h, half, cq)

        def emit_score_tile(h, half, c, cq):
            """[128 keys, 1024 q] scores -> exp -> queue PV."""
            ks = slice(c * 128, (c + 1) * 128)
            sc = work.tile([128, 1024], F32, tag="wk", name="sc")
            for j in range(2):
                qs = slice(half * 1024 + j * 512, half * 1024 + (j + 1) * 512)
                nc.tensor.matmul(
                    sc[:, j * 512 : (j + 1) * 512],
                    lhsT=kd[h][:, ks],
                    rhs=qd[h][:, qs],
                    start=True,
                    stop=True,
                )
            pt = p_pool.tile([128, 1024], F16, tag="pt")
            nc.scalar.activation(
                pt[:], sc[:], EXP, bias=mask_sb[:, c : c + 1], scale=1.0
            )
            pv_q.append((h, half, c, cq, pt))
            if c == KT - 1:
                # eager drain at half boundaries: the normalize chain
                # (whose ctx reads gate slot reuse / the output projection)
                # starts several tiles earlier
                while len(pv_q) > 1:
                    pop_pv()
            elif len(pv_q) > PV_LAG:
                pop_pv()

        def emit_normalize(h, half, cq):
            """denom -> recip -> broadcast -> mult, stage-major over the
            half's two 512-col ctx quarters."""
            base = half * 1024
            if h == 0:
                dst = ctx01[0:64, :]
            elif h == 1:
                dst = ctx_tmp[:]
            else:
                dst = ctx2d[0:64, :]
            denoms, recips, rbcs = [], [], []
            for j in range(2):
                denom = norm_pool.tile([1, 512], F32, tag="denom")
                if h == 2 and half == 1:
                    # ACT is idle after the final exp; keep DVE free for
                    # the tail's output casts
                    nc.scalar.copy(denom[:], cq[j][64:65, :])
                else:
                    nc.vector.tensor_copy(denom[:], cq[j][64:65, :])
                denoms.append(denom)
            for j in range(2):
                recip = norm_pool.tile([1, 512], F32, tag="recip")
                nc.vector.reciprocal_approx_fast(recip[:], denoms[j][:])
                recips.append(recip)
            for j in range(2):
                rbc = norm_pool.tile([64, 512], F32, tag="rbc")
                nc.gpsimd.partition_broadcast(rbc[:], recips[j][:])
                rbcs.append(rbc)
            for j in range(2):
                dcols = slice(base + j * 512, base + (j + 1) * 512)
                nc.vector.tensor_tensor(
                    dst[:, dcols], cq[j][0:64, :], rbcs[j][:], MULT
                )
            if h == 1:
                nc.gpsimd.dma_start(
                    ctx01[64:128, base : base + 1024], ctx_tmp[:, base : base + 1024]
                )
            elif h == 2:
                nc.gpsimd.dma_start(
                    ctx2d[64:128, base : base + 1024], ctx2d[0:64, base : base + 1024]
                )

        def emit_outproj(qt, po, cast_eng, dma_eng):
            """po = ctx01^T Wo01 + ctx2d^T Wo2dup for one 128-row q-tile."""
            qs = slice(qt * 128, (qt + 1) * 128)
            # matmul dest is capped at 512 fp32 elements (one PSUM bank)
            for ns, ne in ((0, 512), (512, 768)):
                nc.tensor.matmul(
                    po[:, ns:ne],
                    lhsT=ctx01[:, qs],
                    rhs=wo_sb[:, ns:ne],
                    start=True,
                    stop=False,
                )
                nc.tensor.matmul(
                    po[:, ns:ne],
                    lhsT=ctx2d[:, qs],
                    rhs=wo2d[:, ns:ne],
                    start=False,
                    stop=True,
                )
            ob = ob_pool.tile([128, H], F16, tag="ob")
            if cast_eng == "v":
                nc.vector.tensor_copy(ob[:], po[:])
            elif cast_eng == "a":
                nc.scalar.copy(ob[:], po[:])
            else:
                nc.vector.tensor_copy(ob[:, 0:384], po[:, 0:384])
                nc.scalar.copy(ob[:, 384:768], po[:, 384:768])
            dma_eng.dma_start(out.ap()[qs, :], ob[:])

        # ---------------- emission schedule ----------------
        # Lead-in: Q0/K0/Q1 projections interleaved per h-chunk (each mm is
        # gated by its xt-chunk DMA, so all three tiles finish ~together at
        # xt-complete). Q1 borrows a then-idle ctx-pool PSUM slot.
        pq1 = ctx_pool.tile([128, 512], F32, tag="ctx", name="pq1")
        pq0 = work.tile([128, 512], F32, tag="wk", name="pq0")
        pk0 = work.tile([128, 512], F32, tag="wk", name="pk0")
        lead = ((pq0, wq_sb, 0), (pk0, wk_sb, 0), (pq1, wq_sb, 1))
        for hc in range(HC):
            for pt_, w_sb, qt in lead:
                nc.tensor.matmul(
                    pt_[:],
                    lhsT=w_sb[:, hc, 0:128],
                    rhs=xt_sb[:, hc, qt * 512 : (qt + 1) * 512],
                    start=(hc == 0),
                    stop=(hc == HC - 1),
                )

        for pt_, dst, b_sb, qt in (
            (pq0, qd, bq_sb, 0),
            (pk0, kd, bk_sb, 0),
            (pq1, qd, bq_sb, 1),
        ):
            qs = slice(qt * 512, (qt + 1) * 512)
            nc.vector.tensor_scalar(
                dst[0][0:64, qs], pt_[0:64, :], b_sb[0:64, 0:1], None, ADD
            )
            nc.vector.tensor_scalar(
                dst[1][64:128, qs], pt_[64:128, :], b_sb[64:128, 0:1], None, ADD
            )
            # sync queue: gpsimd is still streaming xt chunks 2-3 here
            nc.sync.dma_start(dst[0][64:128, qs], dst[0][0:64, qs])
            nc.sync.dma_start(dst[1][0:64, qs], dst[1][64:128, qs])

        def ctx_quarters(name):
            return (
                ctx_pool.tile([65, 512], F32, tag="ctx", name=f"{name}q0"),
                ctx_pool.tile([65, 512], F32, tag="ctx", name=f"{name}q1"),
            )
        # Strictly half-major phases (h, half); each holds its two ctx
        # quarter tiles. Fillers: head 0 half 0 carries V + the remaining
        # Q/K projections; head 1 half 1 carries head-2's Q/K projections;
        # head 2 half 1 carries the output projection for q-tiles 0-7.
        h0_pre = {3: 1, 6: 2, 10: 3}  # K-tile qt before its first use
        h0_post = {0: 2, 1: 3}  # Q-tile qt right after c0/c1
        cq = ctx_quarters("c00")
        for c in range(KT):
            if c in h0_pre:
                emit_qk(wk_sb, kd, bk_sb, h0_pre[c], False)
            emit_score_tile(0, 0, c, cq)
            emit_v(c)
            if c in h0_post:
                emit_qk(wq_sb, qd, bq_sb, h0_post[c], False)

        cq = ctx_quarters("c01")
        for c in range(KT):
            emit_score_tile(0, 1, c, cq)

        cq = ctx_quarters("c10")
        for c in range(KT):
            emit_score_tile(1, 0, c, cq)

        cq = ctx_quarters("c11")
        for c in range(KT):
            if c in (1, 5, 9, 13):
                emit_qk(wb2_sb, None, bq_sb, (c - 1) // 4, True)
            emit_score_tile(1, 1, c, cq)

        cq = ctx_quarters("c20")
        for c in range(KT):
            emit_score_tile(2, 0, c, cq)

        cq = ctx_quarters("c21")
        po_a = None
        for c in range(KT):
            emit_score_tile(2, 1, c, cq)
            if c == 7:
                po_a = work.tile([128, H], F32, tag="wk", name="po_a")
            if c >= 8:
                emit_outproj(
                    c - 8,
                    po_a,
                    "v" if c < 14 else "s",
                    nc.sync if c % 2 == 0 else nc.gpsimd,
                )
        while pv_q:
            pop_pv()

        # tail: q-tiles 8-15, rotating po through freed work-ring slots
        po_b = work.tile([128, H], F32, tag="wk", name="po_b")
        po_c = work.tile([128, H], F32, tag="wk", name="po_c")
        for i, qt in enumerate(range(8, KT)):
            po = (po_b, po_c, po_a)[i % 3]
            emit_outproj(qt, po, "s", nc.sync if i % 2 else nc.scalar)
        work.release()
        ctx_pool.release()


_NC_CACHE = None


def _get_nc():
    global _NC_CACHE
    if _NC_CACHE is None:
        _NC_CACHE = build_kernel()
    return _NC_CACHE


def _pack_w(w):
    """[768, 192] -> [128, 6*192] with row p = concat_c w[c*128+p, :]."""
    return np.ascontiguousarray(
        w.reshape(HC, 128, D3).transpose(1, 0, 2).reshape(128, HC * D3)
    )


def make_in_maps(hidden_states, attention_mask, Wq, bq, Wk, bk, Wv, bv, Wo, bo):
    hidden_states = np.asarray(hidden_states, np.float32)
    attention_mask = np.asarray(attention_mask, np.float32)
    Wq = np.asarray(Wq, np.float32)
    Wk = np.asarray(Wk, np.float32)
    Wv = np.asarray(Wv, np.float32)
    Wo = np.asarray(Wo, np.float32)
    bq = np.asarray(bq, np.float32)
    bk = np.asarray(bk, np.float32)
    bv = np.asarray(bv, np.float32)

    scale = 0.5 / np.sqrt(np.float32(HD))  # extra 1/2: scores use dup-row K=128
    in_maps = []
    for core in range(N_CORES):
        b, g = divmod(core, 4)
        cols = slice(D3 * g, D3 * (g + 1))
        bq_s = (bq[cols] * scale).astype(np.float32)
        bk_s = bk[cols].astype(np.float32)
        bq_pack = np.zeros((2, 128), np.float32)
        bq_pack[0] = bq_s[0:128]
        bq_pack[1, 0:64] = bq_s[128:192]
        bq_pack[1, 64:128] = bk_s[128:192]
        bk_pack = np.zeros((2, 128), np.float32)
        bk_pack[0] = bk_s[0:128]

        # xt partition-major: row p = concat_c X^T[c*128+p, :]
        xtp = (
            np.ascontiguousarray(hidden_states[b].T)
            .astype(np.float16)
            .reshape(HC, 128, S)
            .transpose(1, 0, 2)
            .reshape(128, HC * S)
        )
        wq_p = _pack_w((Wq[:, cols] * scale).astype(np.float16))
        wk_p = _pack_w(Wk[:, cols].astype(np.float16))
        wv_p = _pack_w(Wv[:, cols].astype(np.float16))
        wb2_p = (
            np.concatenate(
                [Wq[:, cols][:, 128:192] * scale, Wk[:, cols][:, 128:192]], axis=1
            )
            .astype(np.float16)
            .reshape(HC, 128, 128)
            .transpose(1, 0, 2)
            .reshape(128, HC * 128)
        )
        # wv|wb2 interleaved per h-chunk: [128, HC, 192+128]
        wvb_p = np.concatenate(
            [wv_p.reshape(128, HC, D3), wb2_p.reshape(128, HC, 128)], axis=2
        ).reshape(128, HC * (D3 + 128))
        wo2h = (Wo[cols, :][128:192] * 0.5).astype(np.float16)
        woo = np.concatenate(
            [
                Wo[cols, :][0:128].astype(np.float16),
                np.concatenate([wo2h, wo2h], axis=0),
            ],
            axis=1,
        )
        bqkm_p = np.concatenate(
            [
                bq_pack.T,
                bk_pack.T,
                attention_mask[b, 0, 0, :].reshape(KT, 128).T,
            ],
            axis=1,
        ).astype(np.float32)
        in_maps.append(
            {
                "xt": np.ascontiguousarray(xtp),
                "wqk": np.ascontiguousarray(np.concatenate([wq_p, wk_p], axis=1)),
                "wvb": np.ascontiguousarray(wvb_p),
                "wo": np.ascontiguousarray(woo),
                "bqkm": np.ascontiguousarray(bqkm_p),
                "bv": bv[cols].reshape(1, D3).astype(np.float16),
            }
        )
    return in_maps


def assemble_out(results, bo):
    out = np.zeros((B, S, H), np.float32)
    for core in range(N_CORES):
        b = core // 4
        out[b] += results[core]["out"].astype(np.float32)
    out += np.asarray(bo, np.float32)
    return out


def kernel(hidden_states, attention_mask, Wq, bq, Wk, bk, Wv, bv, Wo, bo):
    in_maps = make_in_maps(
        hidden_states, attention_mask, Wq, bq, Wk, bk, Wv, bv, Wo, bo
    )
    res = run_bass_kernel_spmd(_get_nc(), in_maps, list(range(N_CORES)))
    return assemble_out(res.results, bo)
